# revision 29
# baseline (speedup 1.0000x reference)
"""AdapCNN block on 8 TRN2 NeuronCores (raw Bass, hand-rolled semaphores).

Strategy (data-parallel over batch, 2 samples per core):
  - The tiny FMN weight-generator MLP (0.8% of FLOPs) runs on host in f32;
    the generated per-sample conv weights are sharded along B to the cores.
  - Each core runs the per-sample 64->64 3x3 VALID conv on its 2 samples.

Conv-as-matmul scheme ("even-base row pairs", 75% PE utilization):
  SBUF x tile xs[s] = [128, 64, 128] bf16: partition (t*64+c) holds row
  2m+t of channel c at free position (m, w) -- every input row stored
  exactly once (halves input DMA vs an overlapping row-pair layout).
  PSUM slot j accumulates BOTH output rows (2j, 2j+1) completely:
  partitions 0:64 = channels of row 2j, 64:128 = row 2j+1.  Per bank of
  4 slots (8 output rows), 6 matmuls (3 kw x {A,B}):
    mmA (rhs pairs j..j+3, pixel offset kw):
      lhsT[t*64+c, dq*64+o] = [[W_kh0, 0], [W_kh1, W_kh0]]
    mmB (rhs pairs j+1..j+4):
      lhsT = [[W_kh2, W_kh1], [0, W_kh2]]
  so row 2j gets kh0*x[2j]+kh1*x[2j+1] (A) + kh2*x[2j+2] (B), row 2j+1
  gets kh0*x[2j+1] (A) + kh1*x[2j+2]+kh2*x[2j+3] (B).  No cross-slot
  combine: the epilogue is a single bias-add+bf16-convert per bank,
  alternating ACT (activation Identity + per-partition bias) / DVE
  (tensor_scalar add).

Engine/DMA layout (bulk DMAs on the two fast HWDGE rings):
  - sync ring: all x chunk DMAs (12-pair head chunk, then 8-pair; sized
    so every bank's data lands >=2us before the tensor engine needs it)
    plus the odd-parity output DMAs; bias rides gpsimd SWDGE
  - ACT ring: weights first, then the even-parity output DMAs -- one
    per 2-bank group (16 groups), staged in 4 rotating ob tiles of
    [128, 8, 126]; the final group is split per-bank so the last
    transfer is small and starts right after the last epilogue op
  - y is written as parity planes [NS, COUT, 2, 63, OW] so each DMA
    descriptor run is >=1764B contiguous; host interleaves rows back
  - 8 psum tensors of [128, 4, 128] f32 (one 2KB bank each) rotate; the
    tensor engine waits for bank g-8's epilogue before reuse
  - DVE memsets a zero tile then 9 warm-up matmuls flip the PE HAM
    clock gate to 2.4 GHz while the first input DMAs are in flight
    (any early-stream PE idle >3.4us would re-throttle the clock)
  - compute dtype bf16 (PSUM accumulates f32); y written bf16 and
    upconverted to f32 on host (rel err ~2.8e-3 end to end)
"""
import sys

if '/opt/trn_rl_repo' not in sys.path:
    sys.path.insert(0, '/opt/trn_rl_repo')

import numpy as np
import ml_dtypes

B, CIN, COUT, K = 16, 64, 64, 3
H = W = 128
OH = OW = 126
FC, FMN0, FMN1, G = 512, 512, 512, 4
CNN_PARA = CIN * COUT * K * K + COUT
NCORES = 8
NS = B // NCORES          # samples per core
NPAIR = H // 2            # 64 stored pair-rows per sample
NSLOT = OH // 2           # 63 psum slots (2 output rows each) per sample
NBANK = 16                # 15 banks of 4 slots + 1 bank of 3 slots
CH0 = [(0, 12), (12, 20), (20, 28), (28, 36), (36, 44), (44, 52),
       (52, 60), (60, 64)]
CH1 = [(0, 13), (13, 26), (26, 39), (39, 52), (52, 64)]
CHUNKS = [CH0, CH1]       # per-sample x chunk pair-bounds
CBASE = [0, len(CH0)]     # chunk-semaphore index base per sample
NGRP = NS * NBANK // 2    # 16 output groups of 2 banks

_cached = {}


def _build_module():
    import concourse.mybir as mybir
    from concourse import bacc

    f32 = mybir.dt.float32
    bf16 = mybir.dt.bfloat16
    add = mybir.AluOpType.add
    ident = mybir.ActivationFunctionType.Identity

    nc = bacc.Bacc("TRN2", target_bir_lowering=False, debug=False,
                   num_devices=NCORES)
    x_ext = nc.declare_dram_parameter("xe", [NS, 128, NPAIR, W], bf16,
                                      isOutput=False)
    wt_ext = nc.declare_dram_parameter("wt", [128, NS * 3 * 2 * 128], bf16,
                                       isOutput=False)
    b_ext = nc.declare_dram_parameter("bias", [128, NS], f32, isOutput=False)
    y_ext = nc.declare_dram_parameter("y", [NS, COUT, 2, NSLOT, OW], bf16,
                                      isOutput=True)

    xs = [nc.alloc_sbuf_tensor(f"xs{s}", [128, NPAIR, W], bf16).ap()
          for s in range(NS)]
    wt_sb = nc.alloc_sbuf_tensor("wt_sb", [128, NS, 3, 2, 128], bf16).ap()
    bias_sb = nc.alloc_sbuf_tensor("bias_sb", [128, NS], f32).ap()
    wz = nc.alloc_sbuf_tensor("wz", [128, 512], bf16).ap()
    obs = [nc.alloc_sbuf_tensor(f"ob{j}", [128, 8, OW], bf16).ap()
           for j in range(4)]
    pss = [nc.alloc_psum_tensor(f"ps{j}", [128, 4, 128], f32).ap()
           for j in range(8)]

    NB = NS * NBANK                                # 32 bank groups total

    def bank_of(gb):
        s, b = gb // NBANK, gb % NBANK
        nsl = 4 if b < NBANK - 1 else NSLOT - 4 * (NBANK - 1)
        return s, b, nsl

    import contextlib
    with contextlib.ExitStack() as ctx:
        s_xc = [ctx.enter_context(nc.semaphore(f"s_xc{i}"))
                for i in range(len(CH0) + len(CH1))]
        s_ws = ctx.enter_context(nc.semaphore("s_ws"))
        s_wt = ctx.enter_context(nc.semaphore("s_wt"))
        s_b = ctx.enter_context(nc.semaphore("s_b"))
        s_mm = ctx.enter_context(nc.semaphore("s_mm"))
        s_act = ctx.enter_context(nc.semaphore("s_act"))
        s_dve = ctx.enter_context(nc.semaphore("s_dve"))
        s_ob = [ctx.enter_context(nc.semaphore(f"s_ob{j}")) for j in range(4)]
        block = ctx.enter_context(nc.Block(no_gpsimd_drain=True))

        def xdma(eng, s, c):
            lo, hi = CHUNKS[s][c]
            eng.dma_start(
                xs[s][:, lo:hi, :], x_ext[s, :, lo:hi, :]
            ).then_inc(s_xc[CBASE[s] + c], 16)

        @block.sync
        def _(sy):
            # tiny bias DMA first: pays the cold-ring startup cost so
            # chunk0's descriptors flow into a warm ring
            sy.dma_start(bias_sb[:], b_ext[:]).then_inc(s_b, 16)
            for c in range(len(CH0)):
                xdma(sy, 0, c)
            for c in range(len(CH1)):
                xdma(sy, 1, c)
            # odd-parity output DMAs (ACT ring carries the even ones);
            # the final group goes per-bank so the very last transfer is
            # small and starts right after the last epilogue op
            for k in range(NGRP):
                s, b0, _ = bank_of(2 * k)
                _, b1, nsl1 = bank_of(2 * k + 1)
                nt = 4 + nsl1
                r0 = 4 * b0
                sy.wait_ge(s_act, k + 1)
                if k < NGRP - 1:
                    sy.wait_ge(s_dve, k + 1)
                    sy.dma_start(y_ext[s, :, 1, r0:r0 + nt, :],
                                 obs[k % 4][64:128, 0:nt, :]
                                 ).then_inc(s_ob[k % 4], 16)
                else:
                    sy.dma_start(y_ext[s, :, 1, r0:r0 + 4, :],
                                 obs[k % 4][64:128, 0:4, :]
                                 ).then_inc(s_ob[k % 4], 16)
                    sy.wait_ge(s_dve, k + 1)
                    sy.dma_start(y_ext[s, :, 1, r0 + 4:r0 + nt, :],
                                 obs[k % 4][64:128, 4:nt, :]
                                 ).then_inc(s_ob[k % 4], 16)
            for j in range(4):
                sy.wait_ge(s_ob[j], 32 * (NGRP // 4) + (16 if j == 3 else 0))

        @block.tensor
        def _(t):
            t.wait_ge(s_ws, 1)
            for _ in range(9):
                nc.tensor.matmul(pss[0][:, 0:4, 0:OW], wz[:, 0:128],
                                 wz[:, 0:504], start=True, stop=True)
            t.wait_ge(s_wt, 16)
            waited = set()
            for gb in range(NB):
                s, b, nsl = bank_of(gb)
                last_pair = 4 * b + 4 if b < NBANK - 1 else NPAIR - 1
                for c in range(len(CHUNKS[s])):
                    if CHUNKS[s][c][0] <= last_pair:
                        i = CBASE[s] + c
                        if i not in waited:
                            t.wait_ge(s_xc[i], 16)
                            waited.add(i)
                if gb >= 8:
                    pg = gb - 8
                    if pg % 2 == 0:
                        t.wait_ge(s_act, pg // 2 + 1)
                    else:
                        t.wait_ge(s_dve, (pg + 1) // 2)
                ps = pss[gb % 8]
                j0 = 4 * b
                for kw in range(3):
                    nc.tensor.matmul(
                        ps[:, 0:nsl, 0:OW],
                        wt_sb[:, s, kw, 0, :],
                        xs[s][:, j0:j0 + nsl, kw:kw + OW],
                        start=(kw == 0), stop=False)
                    mm = nc.tensor.matmul(
                        ps[:, 0:nsl, 0:OW],
                        wt_sb[:, s, kw, 1, :],
                        xs[s][:, j0 + 1:j0 + 1 + nsl, kw:kw + OW],
                        start=False, stop=(kw == 2))
                    if kw == 2:
                        mm.then_inc(s_mm, 1)

        @block.scalar
        def _(sc):
            sc.dma_start(wt_sb.rearrange("p s k a m -> p (s k a m)"),
                         wt_ext[:]).then_inc(s_wt, 16)
            sc.wait_ge(s_b, 16)
            for k in range(NGRP):
                s, b0, _ = bank_of(2 * k)
                _, b1, nsl1 = bank_of(2 * k + 1)
                nt = 4 + nsl1
                r0 = 4 * b0
                ob = obs[k % 4]
                if k >= 4:
                    sc.wait_ge(s_ob[k % 4], 32 * (k // 4))
                sc.wait_ge(s_mm, 2 * k + 1)
                nc.scalar.activation(
                    ob[:, 0:4, :],
                    pss[(2 * k) % 8][:, 0:4, 0:OW],
                    ident, bias=bias_sb[:, s:s + 1],
                ).then_inc(s_act, 1)
                if k < NGRP - 1:
                    sc.wait_ge(s_dve, k + 1)
                    sc.dma_start(y_ext[s, :, 0, r0:r0 + nt, :],
                                 ob[0:64, 0:nt, :]).then_inc(s_ob[k % 4], 16)
                else:
                    sc.dma_start(y_ext[s, :, 0, r0:r0 + 4, :],
                                 ob[0:64, 0:4, :]).then_inc(s_ob[k % 4], 16)
                    sc.wait_ge(s_dve, k + 1)
                    sc.dma_start(y_ext[s, :, 0, r0 + 4:r0 + nt, :],
                                 ob[0:64, 4:nt, :]).then_inc(s_ob[k % 4], 16)
            for j in range(4):
                sc.wait_ge(s_ob[j], 32 * (NGRP // 4) + (16 if j == 3 else 0))

        @block.vector
        def _(v):
            v.memset(wz[:], 0.0).then_inc(s_ws, 1)
            v.wait_ge(s_b, 16)
            for k in range(NGRP):
                s, b1, nsl1 = bank_of(2 * k + 1)
                if k >= 4:
                    v.wait_ge(s_ob[k % 4], 32 * (k // 4))
                v.wait_ge(s_mm, 2 * k + 2)
                nc.vector.tensor_scalar(
                    obs[k % 4][:, 4:4 + nsl1, :],
                    pss[(2 * k + 1) % 8][:, 0:nsl1, 0:OW],
                    bias_sb[:, s:s + 1], None, add,
                ).then_inc(s_dve, 1)

    nc.compile()
    return nc


def _fmn_host(fc_in, w1, b1, w2, b2, w3, b3):
    h = np.maximum(fc_in @ w1.T + b1, 0.0)
    h = np.maximum(h @ w2.T + b2, 0.0)
    hg = h.reshape(h.shape[0], G, FMN1 // G)
    o = np.einsum('bgi,goi->bgo', hg, w3,
                  dtype=np.float32).reshape(h.shape[0], -1) + b3
    return np.maximum(o, 0.0)


def _prep_inputs(x, fc_in, w1, b1, w2, b2, w3, b3):
    wb = _fmn_host(fc_in, w1, b1, w2, b2, w3, b3)          # [B, CNN_PARA]
    weight = wb[:, :-COUT].reshape(B, COUT, CIN, K, K)
    bias = wb[:, -COUT:]                                   # [B, COUT]

    # lhsT blocks: Wc[s, kw, kh, c, o] = weight[s, o, c, kh, kw]
    Wc = weight.transpose(0, 4, 3, 2, 1)
    A = np.zeros((B, 3, 128, 128), np.float32)
    Bm = np.zeros((B, 3, 128, 128), np.float32)
    A[:, :, 0:64, 0:64] = Wc[:, :, 0]
    A[:, :, 64:128, 0:64] = Wc[:, :, 1]
    A[:, :, 64:128, 64:128] = Wc[:, :, 0]
    Bm[:, :, 0:64, 0:64] = Wc[:, :, 2]
    Bm[:, :, 0:64, 64:128] = Wc[:, :, 1]
    Bm[:, :, 64:128, 64:128] = Wc[:, :, 2]
    lhsT = np.stack([A, Bm], axis=2)                       # [B, 3, 2, k, m]
    lhsT = lhsT.astype(ml_dtypes.bfloat16)
    lhsT = lhsT.transpose(3, 0, 1, 2, 4)                   # [128, B, 3, 2, m]

    # x tile: partition t*64+c holds rows 2m+t
    xb = x.astype(ml_dtypes.bfloat16)                      # [B, 64, 128, 128]
    xe = xb.reshape(B, CIN, NPAIR, 2, W).transpose(0, 3, 1, 2, 4)
    xe = np.ascontiguousarray(xe.reshape(B, 128, NPAIR, W))

    in_maps = []
    for c in range(NCORES):
        s0 = NS * c
        in_maps.append({
            "xe": np.ascontiguousarray(xe[s0:s0 + NS]),
            "wt": np.ascontiguousarray(
                lhsT[:, s0:s0 + NS].reshape(128, NS * 3 * 2 * 128)),
            "bias": np.ascontiguousarray(
                np.tile(bias[s0:s0 + NS].T, (2, 1))),      # [128, NS]
        })
    return in_maps


def kernel(x, fc_in, w1, b1, w2, b2, w3, b3, splits):
    from concourse.bass_utils import run_bass_kernel_spmd

    x = np.asarray(x, np.float32)
    args = [np.asarray(a, np.float32)
            for a in (fc_in, w1, b1, w2, b2, w3, b3)]
    in_maps = _prep_inputs(x, *args)

    if 'nc' not in _cached:
        _cached['nc'] = _build_module()
    nc = _cached['nc']

    res = run_bass_kernel_spmd(nc, in_maps, core_ids=list(range(NCORES)))

    out = np.empty((B * COUT, OH, OW), np.float32)
    for c in range(NCORES):
        y = res.results[c]["y"]                # [NS, COUT, 2, NSLOT, OW]
        y = np.asarray(y, np.float32).transpose(0, 1, 3, 2, 4)
        out[NS * COUT * c:NS * COUT * (c + 1)] = \
            y.reshape(NS * COUT, OH, OW)
    return out.reshape(1, B * COUT, 1, OH, OW)


# revision 30
# speedup vs baseline: 1.0119x; 1.0119x over previous
"""AdapCNN block on 8 TRN2 NeuronCores (raw Bass, hand-rolled semaphores).

Strategy (data-parallel over batch, 2 samples per core):
  - The tiny FMN weight-generator MLP (0.8% of FLOPs) runs on host in f32;
    the generated per-sample conv weights are sharded along B to the cores.
  - Each core runs the per-sample 64->64 3x3 VALID conv on its 2 samples.

Conv-as-matmul scheme ("even-base row pairs", 75% PE utilization):
  SBUF x tile xs[s] = [128, 64, 128] bf16: partition (t*64+c) holds row
  2m+t of channel c at free position (m, w) -- every input row stored
  exactly once (halves input DMA vs an overlapping row-pair layout).
  PSUM slot j accumulates BOTH output rows (2j, 2j+1) completely:
  partitions 0:64 = channels of row 2j, 64:128 = row 2j+1.  Per bank of
  4 slots (8 output rows), 6 matmuls (3 kw x {A,B}):
    mmA (rhs pairs j..j+3, pixel offset kw):
      lhsT[t*64+c, dq*64+o] = [[W_kh0, 0], [W_kh1, W_kh0]]
    mmB (rhs pairs j+1..j+4):
      lhsT = [[W_kh2, W_kh1], [0, W_kh2]]
  so row 2j gets kh0*x[2j]+kh1*x[2j+1] (A) + kh2*x[2j+2] (B), row 2j+1
  gets kh0*x[2j+1] (A) + kh1*x[2j+2]+kh2*x[2j+3] (B).  No cross-slot
  combine: the epilogue is a single bias-add+bf16-convert per bank,
  alternating ACT (activation Identity + per-partition bias) / DVE
  (tensor_scalar add).

Engine/DMA layout (bulk DMAs on the two fast HWDGE rings):
  - sync ring: all x chunk DMAs (12-pair head chunk, then 8-pair; sized
    so every bank's data lands >=2us before the tensor engine needs it)
    plus the odd-parity output DMAs; bias rides gpsimd SWDGE
  - ACT ring: weights first, then the even-parity output DMAs -- one
    per 2-bank group (16 groups), staged in 4 rotating ob tiles of
    [128, 8, 126]; the final group is split per-bank so the last
    transfer is small and starts right after the last epilogue op
  - y is written as parity planes [NS, COUT, 2, 63, OW] so each DMA
    descriptor run is >=1764B contiguous; host interleaves rows back
  - 8 psum tensors of [128, 4, 128] f32 (one 2KB bank each) rotate; the
    tensor engine waits for bank g-8's epilogue before reuse
  - DVE memsets a zero tile then 9 warm-up matmuls flip the PE HAM
    clock gate to 2.4 GHz while the first input DMAs are in flight
    (any early-stream PE idle >3.4us would re-throttle the clock)
  - compute dtype bf16 (PSUM accumulates f32); y written bf16 and
    upconverted to f32 on host (rel err ~2.8e-3 end to end)
"""
import sys

if '/opt/trn_rl_repo' not in sys.path:
    sys.path.insert(0, '/opt/trn_rl_repo')

import numpy as np
import ml_dtypes

B, CIN, COUT, K = 16, 64, 64, 3
H = W = 128
OH = OW = 126
FC, FMN0, FMN1, G = 512, 512, 512, 4
CNN_PARA = CIN * COUT * K * K + COUT
NCORES = 8
NS = B // NCORES          # samples per core
NPAIR = H // 2            # 64 stored pair-rows per sample
NSLOT = OH // 2           # 63 psum slots (2 output rows each) per sample
NBANK = 16                # 15 banks of 4 slots + 1 bank of 3 slots
CH0 = [(0, 12), (12, 20), (20, 28), (28, 36), (36, 44), (44, 52),
       (52, 60), (60, 64)]
CH1 = [(0, 13), (13, 26), (26, 39), (39, 52), (52, 64)]
CHUNKS = [CH0, CH1]       # per-sample x chunk pair-bounds
CBASE = [0, len(CH0)]     # chunk-semaphore index base per sample
NGRP = NS * NBANK // 2    # 16 output groups of 2 banks

_cached = {}


def _build_module():
    import concourse.mybir as mybir
    from concourse import bacc

    f32 = mybir.dt.float32
    bf16 = mybir.dt.bfloat16
    add = mybir.AluOpType.add
    ident = mybir.ActivationFunctionType.Identity

    nc = bacc.Bacc("TRN2", target_bir_lowering=False, debug=False,
                   num_devices=NCORES)
    x_ext = nc.declare_dram_parameter("xe", [NS, 128, NPAIR, W], bf16,
                                      isOutput=False)
    wt_ext = nc.declare_dram_parameter("wt", [128, NS * 3 * 2 * 128], bf16,
                                       isOutput=False)
    b_ext = nc.declare_dram_parameter("bias", [128, NS], f32, isOutput=False)
    y_ext = nc.declare_dram_parameter("y", [NS, COUT, 2, NSLOT, OW], bf16,
                                      isOutput=True)

    xs = [nc.alloc_sbuf_tensor(f"xs{s}", [128, NPAIR, W], bf16).ap()
          for s in range(NS)]
    wt_sb = nc.alloc_sbuf_tensor("wt_sb", [128, NS, 3, 2, 128], bf16).ap()
    bias_sb = nc.alloc_sbuf_tensor("bias_sb", [128, NS], f32).ap()
    wz = nc.alloc_sbuf_tensor("wz", [128, 512], bf16).ap()
    obs = [nc.alloc_sbuf_tensor(f"ob{j}", [128, 8, OW], bf16).ap()
           for j in range(4)]
    pss = [nc.alloc_psum_tensor(f"ps{j}", [128, 4, 128], f32).ap()
           for j in range(8)]

    NB = NS * NBANK                                # 32 bank groups total

    def bank_of(gb):
        s, b = gb // NBANK, gb % NBANK
        nsl = 4 if b < NBANK - 1 else NSLOT - 4 * (NBANK - 1)
        return s, b, nsl

    import contextlib
    with contextlib.ExitStack() as ctx:
        s_xc = [ctx.enter_context(nc.semaphore(f"s_xc{i}"))
                for i in range(len(CH0) + len(CH1))]
        s_ws = ctx.enter_context(nc.semaphore("s_ws"))
        s_wt = ctx.enter_context(nc.semaphore("s_wt"))
        s_b = ctx.enter_context(nc.semaphore("s_b"))
        s_mm = ctx.enter_context(nc.semaphore("s_mm"))
        s_act = ctx.enter_context(nc.semaphore("s_act"))
        s_dve = ctx.enter_context(nc.semaphore("s_dve"))
        s_ob = [ctx.enter_context(nc.semaphore(f"s_ob{j}")) for j in range(4)]
        block = ctx.enter_context(nc.Block(no_gpsimd_drain=True))

        def xdma(eng, s, c):
            lo, hi = CHUNKS[s][c]
            eng.dma_start(
                xs[s][:, lo:hi, :], x_ext[s, :, lo:hi, :]
            ).then_inc(s_xc[CBASE[s] + c], 16)

        @block.gpsimd
        def _(g):
            g.dma_start(bias_sb[:], b_ext[:]).then_inc(s_b, 16)

        @block.sync
        def _(sy):
            for c in range(len(CH0)):
                xdma(sy, 0, c)
            for c in range(len(CH1)):
                xdma(sy, 1, c)
            # odd-parity output DMAs (ACT ring carries the even ones);
            # the final group goes per-bank so the very last transfer is
            # small and starts right after the last epilogue op
            for k in range(NGRP):
                s, b0, _ = bank_of(2 * k)
                _, b1, nsl1 = bank_of(2 * k + 1)
                nt = 4 + nsl1
                r0 = 4 * b0
                sy.wait_ge(s_act, k + 1)
                if k < NGRP - 1:
                    sy.wait_ge(s_dve, k + 1)
                    sy.dma_start(y_ext[s, :, 1, r0:r0 + nt, :],
                                 obs[k % 4][64:128, 0:nt, :]
                                 ).then_inc(s_ob[k % 4], 16)
                else:
                    sy.dma_start(y_ext[s, :, 1, r0:r0 + 4, :],
                                 obs[k % 4][64:128, 0:4, :]
                                 ).then_inc(s_ob[k % 4], 16)
                    sy.wait_ge(s_dve, k + 1)
                    sy.dma_start(y_ext[s, :, 1, r0 + 4:r0 + nt, :],
                                 obs[k % 4][64:128, 4:nt, :]
                                 ).then_inc(s_ob[k % 4], 16)
            for j in range(4):
                sy.wait_ge(s_ob[j], 32 * (NGRP // 4) + (16 if j == 3 else 0))

        @block.tensor
        def _(t):
            t.wait_ge(s_ws, 1)
            for _ in range(9):
                nc.tensor.matmul(pss[0][:, 0:4, 0:OW], wz[:, 0:128],
                                 wz[:, 0:504], start=True, stop=True)
            t.wait_ge(s_wt, 16)
            waited = set()
            for gb in range(NB):
                s, b, nsl = bank_of(gb)
                last_pair = 4 * b + 4 if b < NBANK - 1 else NPAIR - 1
                for c in range(len(CHUNKS[s])):
                    if CHUNKS[s][c][0] <= last_pair:
                        i = CBASE[s] + c
                        if i not in waited:
                            t.wait_ge(s_xc[i], 16)
                            waited.add(i)
                if gb >= 8:
                    pg = gb - 8
                    if pg % 2 == 0:
                        t.wait_ge(s_act, pg // 2 + 1)
                    else:
                        t.wait_ge(s_dve, (pg + 1) // 2)
                ps = pss[gb % 8]
                j0 = 4 * b
                for kw in range(3):
                    nc.tensor.matmul(
                        ps[:, 0:nsl, 0:OW],
                        wt_sb[:, s, kw, 0, :],
                        xs[s][:, j0:j0 + nsl, kw:kw + OW],
                        start=(kw == 0), stop=False)
                    mm = nc.tensor.matmul(
                        ps[:, 0:nsl, 0:OW],
                        wt_sb[:, s, kw, 1, :],
                        xs[s][:, j0 + 1:j0 + 1 + nsl, kw:kw + OW],
                        start=False, stop=(kw == 2))
                    if kw == 2:
                        mm.then_inc(s_mm, 1)

        @block.scalar
        def _(sc):
            sc.dma_start(wt_sb.rearrange("p s k a m -> p (s k a m)"),
                         wt_ext[:]).then_inc(s_wt, 16)
            sc.wait_ge(s_b, 16)
            for k in range(NGRP):
                s, b0, _ = bank_of(2 * k)
                _, b1, nsl1 = bank_of(2 * k + 1)
                nt = 4 + nsl1
                r0 = 4 * b0
                ob = obs[k % 4]
                if k >= 4:
                    sc.wait_ge(s_ob[k % 4], 32 * (k // 4))
                sc.wait_ge(s_mm, 2 * k + 1)
                nc.scalar.activation(
                    ob[:, 0:4, :],
                    pss[(2 * k) % 8][:, 0:4, 0:OW],
                    ident, bias=bias_sb[:, s:s + 1],
                ).then_inc(s_act, 1)
                if k < NGRP - 1:
                    sc.wait_ge(s_dve, k + 1)
                    sc.dma_start(y_ext[s, :, 0, r0:r0 + nt, :],
                                 ob[0:64, 0:nt, :]).then_inc(s_ob[k % 4], 16)
                else:
                    sc.dma_start(y_ext[s, :, 0, r0:r0 + 4, :],
                                 ob[0:64, 0:4, :]).then_inc(s_ob[k % 4], 16)
                    sc.wait_ge(s_dve, k + 1)
                    sc.dma_start(y_ext[s, :, 0, r0 + 4:r0 + nt, :],
                                 ob[0:64, 4:nt, :]).then_inc(s_ob[k % 4], 16)
            for j in range(4):
                sc.wait_ge(s_ob[j], 32 * (NGRP // 4) + (16 if j == 3 else 0))

        @block.vector
        def _(v):
            v.memset(wz[:], 0.0).then_inc(s_ws, 1)
            v.wait_ge(s_b, 16)
            for k in range(NGRP):
                s, b1, nsl1 = bank_of(2 * k + 1)
                if k >= 4:
                    v.wait_ge(s_ob[k % 4], 32 * (k // 4))
                v.wait_ge(s_mm, 2 * k + 2)
                nc.vector.tensor_scalar(
                    obs[k % 4][:, 4:4 + nsl1, :],
                    pss[(2 * k + 1) % 8][:, 0:nsl1, 0:OW],
                    bias_sb[:, s:s + 1], None, add,
                ).then_inc(s_dve, 1)

    nc.compile()
    return nc


def _fmn_host(fc_in, w1, b1, w2, b2, w3, b3):
    h = np.maximum(fc_in @ w1.T + b1, 0.0)
    h = np.maximum(h @ w2.T + b2, 0.0)
    hg = h.reshape(h.shape[0], G, FMN1 // G)
    o = np.einsum('bgi,goi->bgo', hg, w3,
                  dtype=np.float32).reshape(h.shape[0], -1) + b3
    return np.maximum(o, 0.0)


def _prep_inputs(x, fc_in, w1, b1, w2, b2, w3, b3):
    wb = _fmn_host(fc_in, w1, b1, w2, b2, w3, b3)          # [B, CNN_PARA]
    weight = wb[:, :-COUT].reshape(B, COUT, CIN, K, K)
    bias = wb[:, -COUT:]                                   # [B, COUT]

    # lhsT blocks: Wc[s, kw, kh, c, o] = weight[s, o, c, kh, kw]
    Wc = weight.transpose(0, 4, 3, 2, 1)
    A = np.zeros((B, 3, 128, 128), np.float32)
    Bm = np.zeros((B, 3, 128, 128), np.float32)
    A[:, :, 0:64, 0:64] = Wc[:, :, 0]
    A[:, :, 64:128, 0:64] = Wc[:, :, 1]
    A[:, :, 64:128, 64:128] = Wc[:, :, 0]
    Bm[:, :, 0:64, 0:64] = Wc[:, :, 2]
    Bm[:, :, 0:64, 64:128] = Wc[:, :, 1]
    Bm[:, :, 64:128, 64:128] = Wc[:, :, 2]
    lhsT = np.stack([A, Bm], axis=2)                       # [B, 3, 2, k, m]
    lhsT = lhsT.astype(ml_dtypes.bfloat16)
    lhsT = lhsT.transpose(3, 0, 1, 2, 4)                   # [128, B, 3, 2, m]

    # x tile: partition t*64+c holds rows 2m+t
    xb = x.astype(ml_dtypes.bfloat16)                      # [B, 64, 128, 128]
    xe = xb.reshape(B, CIN, NPAIR, 2, W).transpose(0, 3, 1, 2, 4)
    xe = np.ascontiguousarray(xe.reshape(B, 128, NPAIR, W))

    in_maps = []
    for c in range(NCORES):
        s0 = NS * c
        in_maps.append({
            "xe": np.ascontiguousarray(xe[s0:s0 + NS]),
            "wt": np.ascontiguousarray(
                lhsT[:, s0:s0 + NS].reshape(128, NS * 3 * 2 * 128)),
            "bias": np.ascontiguousarray(
                np.tile(bias[s0:s0 + NS].T, (2, 1))),      # [128, NS]
        })
    return in_maps


def kernel(x, fc_in, w1, b1, w2, b2, w3, b3, splits):
    from concourse.bass_utils import run_bass_kernel_spmd

    x = np.asarray(x, np.float32)
    args = [np.asarray(a, np.float32)
            for a in (fc_in, w1, b1, w2, b2, w3, b3)]
    in_maps = _prep_inputs(x, *args)

    if 'nc' not in _cached:
        _cached['nc'] = _build_module()
    nc = _cached['nc']

    res = run_bass_kernel_spmd(nc, in_maps, core_ids=list(range(NCORES)))

    out = np.empty((B * COUT, OH, OW), np.float32)
    for c in range(NCORES):
        y = res.results[c]["y"]                # [NS, COUT, 2, NSLOT, OW]
        y = np.asarray(y, np.float32).transpose(0, 1, 3, 2, 4)
        out[NS * COUT * c:NS * COUT * (c + 1)] = \
            y.reshape(NS * COUT, OH, OW)
    return out.reshape(1, B * COUT, 1, OH, OW)
